# revision 1
# baseline (speedup 1.0000x reference)
"""Trainium2 Bass kernel for nn_Encoder_70781061038947 — log-domain pipeline.

Row b's output depends only on its 16 sign bits: log E(t_m) = sum_k b_k d_km
+ C_m is LINEAR in the bits, so one [102x102] matmul per 384-row block
yields log-magnitudes and phases (in cycles) at the 17 roots of unity.
ACT exp/sin reconstruct evals (sin range-reduced via a magic-constant
round on DVE), Parseval gives the per-row norm, and a block-diagonal
inverse-DFT matmul returns coefficients.  Per-eval constants (phase
rotation gamma_m, fp16 residuals, a sigma=16 fp16-headroom shift) are
folded into host tables; ACT calls are phase-ordered by table set.

Sharding: the map depends on x only through its 2^16 sign patterns, so the
device evaluates each distinct pattern once — 8192 patterns per core
(padded to 10368 = 27 blocks of 384 = 3 superblocks of 9), data parallel
across 8 cores.  The host computes the 16-bit pattern index per row
(comparisons + bit-packing only, no float math) and gathers rows from the
device-computed table.
"""

import numpy as np
import ml_dtypes

import concourse.bacc as bacc
import concourse.bass as bass
import concourse.mybir as mybir
import concourse.bass_utils as bass_utils
import concourse.tile as tile

B = 262144
K = 16
NPAT = 1 << 16               # distinct sign patterns
PPC = NPAT // 8              # 8192 patterns per core
M = 17
W = 2 * M                    # 34 f32 per row out
NCORES = 8
RPC = B // NCORES            # 32768 real rows per core
P = 128
TP = 3                       # t' groups per block
BLK = TP * P                 # 384 rows per block
NBLK = 24
R = NBLK * BLK               # 34560 padded rows per core
SB = 8                       # blocks per superblock
NSB = NBLK // SB             # 10
CR = 2 * K + 2               # 34 contraction rows per t' (hi, lo, chi, clo)
CP = TP * CR                 # 102 contraction partitions
FB = TP * M                  # 51 mag (or phase) values per block
FO = TP * W                  # 102 out cols per block
XC = NBLK * P                # 11520 bit-cols per core
OW = 1024                    # padded out cols per superblock (2 psum banks)
SIGMA = 1024.0               # fp16 headroom for A AND A^2
MAGIC = float(1.5 * 2 ** 23)

# psum col map for 9 blocks of 102 f32 in 2 banks of 512
def _pcol(i):
    return 102 * i if i < 5 else 512 + 102 * (i - 5)


_cached = None


def _tables(shuffle_vector: np.ndarray):
    sv = np.asarray(shuffle_vector, dtype=np.float64)
    Rr = np.sqrt(1.0 + np.sin(np.pi / K))
    t = np.exp(2j * np.pi * np.arange(M) / M)
    zp = Rr * np.exp(1j * sv)
    zm = (1.0 / Rr) * np.exp(1j * sv)
    f16 = lambda a: np.asarray(a, np.float16).astype(np.float64)

    dk = np.log((t[None, :] - zp[:, None]) / (t[None, :] - zm[:, None]))  # (K, M)
    Cm = np.sum(np.log(t[None, :] - zm[:, None]), axis=0)                 # (M,)

    dmag = dk.real
    dphase = dk.imag / (2 * np.pi)          # cycles
    cmag = Cm.real - np.log(SIGMA)

    dmag_hi = f16(dmag); dmag_lo = f16(dmag - dmag_hi)
    dph_hi = f16(dphase); dph_lo = f16(dphase - dph_hi)
    cmag_hi = f16(cmag); cmag_lo = f16(cmag - cmag_hi)

    wmag = np.zeros((CR, M))
    wphase = np.zeros((CR, M))
    wmag[:K] = dmag_hi; wmag[K:2 * K] = dmag_lo
    wmag[2 * K] = cmag_hi; wmag[2 * K + 1] = cmag_lo
    wphase[:K] = dph_hi; wphase[K:2 * K] = dph_lo

    wtbl = np.zeros((CP, 2 * FB))            # cols 0..50 mag, 51..101 phase
    for tp in range(TP):
        wtbl[tp * CR:(tp + 1) * CR, tp * M:(tp + 1) * M] = wmag
        wtbl[tp * CR:(tp + 1) * CR, FB + tp * M:FB + (tp + 1) * M] = wphase

    delta = cmag - (f16(cmag_hi) + f16(cmag_lo))
    gamma = np.exp(1j * Cm.imag) * np.exp(delta)
    Wc = np.zeros((M, M), np.complex128)
    for m in range(M):
        for d in range(M):
            Wc[m, d] = np.exp(-2j * np.pi * ((K - d) * m) / M) / M
    Wc = Wc * gamma[:, None]
    W2R = np.zeros((W, W))
    W2R[:M, 0::2] = Wc.real
    W2R[:M, 1::2] = Wc.imag
    W2R[M:, 0::2] = -Wc.imag
    W2R[M:, 1::2] = Wc.real

    w2r3 = np.zeros((FO, FO))
    for tp in range(TP):
        w2r3[tp * W:(tp + 1) * W, tp * W:(tp + 1) * W] = W2R

    return {
        "wtbl": wtbl.astype(np.float16),
        "w2r3": w2r3.astype(np.float16),
        "ident": np.eye(P, dtype=np.float16),
        "cst": np.full((P, 1), np.log(17.0), dtype=np.float32),
        "cmg": np.full((P, 1), MAGIC, dtype=np.float32),
    }


def _build_module():
    f32 = mybir.dt.float32
    f16 = mybir.dt.float16
    bf16 = mybir.dt.bfloat16
    AF = mybir.ActivationFunctionType
    OP = mybir.AluOpType

    nc = bacc.Bacc("TRN2", target_bir_lowering=False, debug=False)
    xdup_d = nc.dram_tensor("xdup", [CP, XC], f16, kind="ExternalInput")
    wtbl_d = nc.dram_tensor("wtbl", [CP, 2 * FB], f16, kind="ExternalInput")
    w2r3_d = nc.dram_tensor("w2r3", [FO, FO], f16, kind="ExternalInput")
    ident_d = nc.dram_tensor("ident", [P, P], f16, kind="ExternalInput")
    cst_d = nc.dram_tensor("cst", [P, 1], f32, kind="ExternalInput")
    cmg_d = nc.dram_tensor("cmg", [P, 1], f32, kind="ExternalInput")
    out_d = nc.dram_tensor("out", [P, NSB * OW], bf16, kind="ExternalOutput")
    out_v = out_d.ap()

    SBW = SB * P                # 1152 bit-cols per superblock
    SBF = SB * FB               # 459
    NT = SB * TP                # 27 row-groups per superblock

    with tile.TileContext(nc) as tc:
        with (
            tc.tile_pool(name="const", bufs=1) as cp,
            tc.tile_pool(name="stage", bufs=1) as st,
            tc.tile_pool(name="work", bufs=3) as wp,
            tc.tile_pool(name="pslp", bufs=2, space="PSUM") as pl,
            tc.tile_pool(name="psod", bufs=1, space="PSUM") as po,
        ):
            # input slices issued first on the SP queue: they are the
            # large payloads gating the first matmuls; consts are small
            s_sb = st.tile([CP, XC], f16, name="s")
            for sb in range(NSB):
                nc.sync.dma_start(
                    out=s_sb[:, sb * SBW:(sb + 1) * SBW],
                    in_=xdup_d.ap()[:, sb * SBW:(sb + 1) * SBW])

            wtbl_sb = cp.tile([CP, 2 * FB], f16)
            nc.sync.dma_start(out=wtbl_sb[:], in_=wtbl_d.ap())
            w2r3_sb = cp.tile([FO, FO], f16)
            nc.sync.dma_start(out=w2r3_sb[:], in_=w2r3_d.ap())
            ident_sb = cp.tile([P, P], f16)
            nc.sync.dma_start(out=ident_sb[:], in_=ident_d.ap())
            cst_sb = cp.tile([P, 1], f32)
            nc.sync.dma_start(out=cst_sb[:], in_=cst_d.ap())
            cmg_sb = cp.tile([P, 1], f32)
            nc.sync.dma_start(out=cmg_sb[:], in_=cmg_d.ap())

            A_all = st.tile([P, NBLK * FB], f16, name="A")
            # negr and rcp interleaved per-superblock: [negr(459) | rcp(459)]
            nr_all = st.tile([P, 2 * NBLK * FB], f16, name="nr")
            S_all = st.tile([P, NBLK * TP], f32, name="S")
            f_all = st.tile([P, NBLK * TP], f32, name="f")
            ln_t = st.tile([P, NBLK * TP], f32, name="lnS")

            B2 = (SB - 5) * FO          # live cols in psum bank 2

            def hole_views(tile_ap, width):
                """[128, 1024]-bank tile -> views [128, 5, width], [128, SB-5, width]."""
                v = tile_ap
                va = v[:, 0:510].rearrange("p (b c) -> p b c", c=FO)
                vb = v[:, 512:512 + B2].rearrange("p (b c) -> p b c", c=FO)
                return va[:, :, 0:width], vb[:, :, 0:width]

            # ---------- PASS A ----------
            for sb in range(NSB):
                lp = pl.tile([P, OW], f32, tag="lp")
                for i in range(SB):
                    nc.tensor.matmul(
                        out=lp[:, _pcol(i):_pcol(i) + FO],
                        lhsT=s_sb[:, (sb * SB + i) * P:(sb * SB + i + 1) * P],
                        rhs=wtbl_sb[:], start=True, stop=True)

                magA, magB = hole_views(lp[:], FB)
                phA = lp[:, 0:510].rearrange("p (b c) -> p b c", c=FO)[:, :, FB:FO]
                phB = lp[:, 512:512 + B2].rearrange("p (b c) -> p b c", c=FO)[:, :, FB:FO]

                # A = exp(Lmag) fp16 (two calls: bank-hole split)
                a_sl = A_all[:, sb * SBF:(sb + 1) * SBF]
                nc.scalar.activation(
                    out=a_sl[:, 0:5 * FB].rearrange("p (b c) -> p b c", c=FB),
                    in_=magA, func=AF.Exp)
                nc.scalar.activation(
                    out=a_sl[:, 5 * FB:SBF].rearrange("p (b c) -> p b c", c=FB),
                    in_=magB, func=AF.Exp)

                # kp = phi + MAGIC on ACT (Identity is in every table set);
                # the f32 add rounds phi to the nearest integer (plus MAGIC).
                kp = wp.tile([P, SBF], f32, tag="kp")
                nc.scalar.activation(
                    out=kp[:, 0:5 * FB].rearrange("p (b c) -> p b c", c=FB),
                    in_=phA, func=AF.Identity, bias=cmg_sb[:])
                nc.scalar.activation(
                    out=kp[:, 5 * FB:SBF].rearrange("p (b c) -> p b c", c=FB),
                    in_=phB, func=AF.Identity, bias=cmg_sb[:])

                # negr = (kp - MAGIC) - phi, fused on DVE; staged fp16
                ng = nr_all[:, 2 * sb * SBF:2 * sb * SBF + SBF]
                nc.vector.scalar_tensor_tensor(
                    out=ng[:, 0:5 * FB].rearrange("p (b c) -> p b c", c=FB),
                    in0=kp[:, 0:5 * FB].rearrange("p (b c) -> p b c", c=FB),
                    scalar=MAGIC, in1=phA, op0=OP.subtract, op1=OP.subtract)
                nc.vector.scalar_tensor_tensor(
                    out=ng[:, 5 * FB:SBF].rearrange("p (b c) -> p b c", c=FB),
                    in0=kp[:, 5 * FB:SBF].rearrange("p (b c) -> p b c", c=FB),
                    scalar=MAGIC, in1=phB, op0=OP.subtract, op1=OP.subtract)

                # rcp = wrap(negr - 0.25) for the cos branch
                nc.vector.add_range_wrap(
                    out=nr_all[:, 2 * sb * SBF + SBF:2 * (sb + 1) * SBF],
                    in_=ng, shift=-0.25, bound=0.5, period=1.0)

                # A^2 in fp16 (sigma=1024 keeps it in range) then reduce
                a2 = wp.tile([P, SBF], f16, tag="a2")
                nc.vector.tensor_tensor(out=a2[:], in0=a_sl, in1=a_sl, op=OP.mult)
                nc.vector.tensor_reduce(
                    out=S_all[:, sb * NT:(sb + 1) * NT],
                    in_=a2[:].rearrange("p (t e) -> p t e", e=M),
                    axis=mybir.AxisListType.X, op=OP.add)

            # f = sqrt(289/S): DVE reciprocal + one ACT Sqrt (single table set)
            with tc.tile_wait_until(0.014):
                nc.vector.reciprocal(out=ln_t[:], in_=S_all[:])
                nc.scalar.activation(
                    out=f_all[:], in_=ln_t[:], func=AF.Sqrt, scale=289.0)

            # ---------- PASS B ----------
            # tile_wait_until keeps pass-B Sin calls from being hoisted into
            # pass A by the scheduler (ACT table-set thrash otherwise).
            for sb in range(NSB):
              with tc.tile_wait_until(0.016 + 0.002 * sb):
                  # one merged Sin over [negr | rcp] -> [s1 | c1]
                  sc = wp.tile([P, 2 * SBF], f16, tag="sc")
                  nc.scalar.activation(
                      out=sc[:], in_=nr_all[:, 2 * sb * SBF:2 * (sb + 1) * SBF],
                      func=AF.Sin, scale=float(-2 * np.pi))
                  s1 = sc[:, 0:SBF]
                  c1 = sc[:, SBF:2 * SBF]

                  # A' = A * f
                  af = wp.tile([P, SBF], f16, tag="af")
                  fv = f_all[:, sb * NT:(sb + 1) * NT].unsqueeze(2).to_broadcast(
                      [P, NT, M])
                  nc.vector.tensor_tensor(
                      out=af[:].rearrange("p (t e) -> p t e", e=M),
                      in0=A_all[:, sb * SBF:(sb + 1) * SBF].rearrange(
                          "p (t e) -> p t e", e=M),
                      in1=fv, op=OP.mult)

                  # E' assembly: [t', re17, im17] per block
                  ee = wp.tile([P, SB * FO], f16, tag="ee")
                  eev = ee[:].rearrange("p (t e) -> p t e", e=W)
                  av = af[:].rearrange("p (t e) -> p t e", e=M)
                  nc.vector.tensor_tensor(
                      out=eev[:, :, 0:M], in0=av,
                      in1=c1.rearrange("p (t e) -> p t e", e=M), op=OP.mult)
                  nc.vector.tensor_tensor(
                      out=eev[:, :, M:W], in0=av,
                      in1=s1.rearrange("p (t e) -> p t e", e=M), op=OP.mult)

                  et = po.tile([FO, SB * P], f16, tag="et")
                  for i in range(SB):
                      nc.tensor.transpose(
                          out=et[:, i * P:(i + 1) * P],
                          in_=ee[:, i * FO:(i + 1) * FO],
                          identity=ident_sb[:])
                  etsb = wp.tile([FO, SB * P], f16, tag="etsb")
                  nc.vector.tensor_copy(out=etsb[:], in_=et[:])

                  op_ps = po.tile([P, OW], f32, tag="op")
                  for i in range(SB):
                      nc.tensor.matmul(
                          out=op_ps[:, _pcol(i):_pcol(i) + FO],
                          lhsT=etsb[:, i * P:(i + 1) * P],
                          rhs=w2r3_sb[:], start=True, stop=True)

                  osb = wp.tile([P, OW], bf16, tag="osb")
                  nc.scalar.copy(out=osb[:], in_=op_ps[:])
                  nc.sync.dma_start(
                      out=out_v[:, sb * OW:(sb + 1) * OW], in_=osb[:])

    nc.compile()
    return nc


def _prep_core(core: int) -> np.ndarray:
    """bits of patterns [8192*core, 8192*(core+1)) -> [102, 3456] fp16."""
    pats = np.arange(core * PPC, (core + 1) * PPC, dtype=np.uint32)
    bits = np.ones((R, K), np.float16)
    bits[:PPC] = ((pats[:, None] >> np.arange(K)[None, :]) & 1)
    v = bits.reshape(NBLK, TP, P, K)
    xt = v.transpose(1, 3, 0, 2)
    xdup = np.ones((TP, CR, NBLK, P), np.float16)
    xdup[:, 0:K] = xt
    xdup[:, K:2 * K] = xt
    return np.ascontiguousarray(xdup.reshape(CP, XC))


def _unpack_core(res: np.ndarray) -> np.ndarray:
    """out [128, 10*1024] bf16 -> (32768, 17) complex64."""
    o = np.asarray(res).astype(np.float32).reshape(P, NSB, OW)
    a = o[:, :, 0:510].reshape(P, NSB, 5, FO)
    b = o[:, :, 512:512 + (SB - 5) * FO].reshape(P, NSB, SB - 5, FO)
    full = np.concatenate([a, b], axis=2)          # (128, 10, 9, 102)
    full = full.reshape(P, NSB, SB, TP, W).transpose(1, 2, 3, 0, 4)
    full = np.ascontiguousarray(full.reshape(R, W)[:PPC])
    return full.view(np.complex64).reshape(PPC, M)


def kernel(x: np.ndarray, shuffle_vector: np.ndarray) -> np.ndarray:
    global _cached
    x = np.asarray(x)
    assert x.shape == (B, K), x.shape

    tabs = _tables(shuffle_vector)
    if _cached is None:
        _cached = _build_module()
    nc = _cached

    idx = ((x > 0).astype(np.uint32) @ (np.uint32(1) << np.arange(K, dtype=np.uint32)))
    in_maps = [{"xdup": _prep_core(i), **tabs} for i in range(NCORES)]
    res = bass_utils.run_bass_kernel_spmd(nc, in_maps, core_ids=list(range(NCORES)))
    table = np.concatenate(
        [_unpack_core(res.results[i]["out"]) for i in range(NCORES)], axis=0)
    return table[idx].astype(np.complex128)



# revision 2
# speedup vs baseline: 1.7477x; 1.7477x over previous
"""Trainium2 Bass kernel for nn_Encoder_70781061038947 — factored-table matmul.

Row b's output depends only on its 16 sign bits, so the device computes a
65536-entry table and the host gathers rows.  The eval vector factorizes:
E(p) = Elo(p & 1023) * Ehi(p >> 10), with both factor tables precomputed on
host in fp64.  Unnormalized coefficients C0 = iDFT(E) are then LINEAR in
Elo with the per-group Ehi folded into the iDFT matrix, so the whole device
kernel is 3 matmul passes: out[102, 1024] = blockdiag(W . Ehi_h for 3
groups)^T @ vstack(LoT x3).  Row norms follow from C0 itself (Parseval), so
normalization happens on host during the gather (any per-group/global
scaling cancels there, which also makes fp16 staging safe).

Sharding: pure data parallel over the 65536 patterns — 8192 patterns
(8 hi-groups of 1024) per core.
"""

import numpy as np

import concourse.bacc as bacc
import concourse.bass as bass
import concourse.mybir as mybir
import concourse.bass_utils as bass_utils
import concourse.tile as tile

B = 262144
K = 16
M = 17
W2 = 2 * M                   # 34 realified rows/cols
LO = 10                      # low bits -> 1024-entry Elo table
NLO = 1 << LO
NHI = 1 << (K - LO)          # 64 hi groups
NCORES = 8
GPC = NHI // NCORES          # 8 hi-groups per core
NPASS = 3                    # 3 groups per matmul pass (3*34=102 rows)
CT = NPASS * W2              # 102
HALF = 512                   # psum bank width in f32

_cached = None


def _tables(shuffle_vector: np.ndarray):
    sv = np.asarray(shuffle_vector, dtype=np.float64)
    R = np.sqrt(1.0 + np.sin(np.pi / K))
    t = np.exp(2j * np.pi * np.arange(M) / M)
    zp = R * np.exp(1j * sv)
    zm = (1.0 / R) * np.exp(1j * sv)

    def factor_table(ks):
        tab = np.ones((1 << len(ks), M), np.complex128)
        for i, k in enumerate(ks):
            bit = (np.arange(1 << len(ks)) >> i) & 1
            tab *= t[None, :] - np.where(bit[:, None] > 0, zp[k], zm[k])
        return tab

    Elo = factor_table(list(range(LO)))          # (1024, 17)
    Ehi = factor_table(list(range(LO, K)))       # (64, 17)

    lo_scale = 2.0 ** np.floor(np.log2(
        2048.0 / np.abs(np.concatenate([Elo.real, Elo.imag])).max()))
    LoT = np.concatenate([Elo.real.T, Elo.imag.T], axis=0) * lo_scale
    rhs3 = np.tile(LoT, (NPASS, 1)).astype(np.float16)   # (102, 1024)

    # c_d = (1/17) sum_m E_m t_m^{-(K-d)}; fold Ehi[h] into the matrix.
    Wc0 = np.exp(-2j * np.pi * np.outer(K - np.arange(M), np.arange(M)) / M).T / M

    def realify(Wc):
        W2R = np.zeros((W2, W2))
        W2R[:M, 0::2] = Wc.real
        W2R[:M, 1::2] = Wc.imag
        W2R[M:, 0::2] = -Wc.imag
        W2R[M:, 1::2] = Wc.real
        return W2R

    lhst = np.zeros((NCORES, CT, NPASS * CT), np.float16)
    for c in range(NCORES):
        for p in range(NPASS):
            for j in range(NPASS):
                g = NPASS * p + j
                if g >= GPC:
                    continue
                h = GPC * c + g
                W2R = realify(Wc0 * Ehi[h][:, None])
                W2R *= 2.0 ** np.floor(np.log2(1.0 / np.abs(W2R).max()))
                lhst[c, j * W2:(j + 1) * W2, p * CT + j * W2:p * CT + (j + 1) * W2] = W2R
    return {"rhs3": rhs3, "lhst": lhst}


def _build_module():
    f32 = mybir.dt.float32
    f16 = mybir.dt.float16

    nc = bacc.Bacc("TRN2", target_bir_lowering=False, debug=False)
    rhs3_d = nc.dram_tensor("rhs3", [CT, NLO], f16, kind="ExternalInput")
    lhst_d = nc.dram_tensor("lhst", [CT, NPASS * CT], f16, kind="ExternalInput")
    out_d = nc.dram_tensor("out", [CT, NPASS * NLO], f16, kind="ExternalOutput")
    out_v = out_d.ap()

    with tile.TileContext(nc) as tc:
        with (
            tc.tile_pool(name="const", bufs=1) as cp,
            tc.tile_pool(name="work", bufs=6) as wp,
            tc.tile_pool(name="ps", bufs=3, space="PSUM") as pl,
        ):
            lhst_sb = cp.tile([CT, NPASS * CT], f16)
            nc.sync.dma_start(out=lhst_sb[:], in_=lhst_d.ap())
            rhs3_sb = cp.tile([CT, NLO], f16)
            nc.sync.dma_start(out=rhs3_sb[:, 0:HALF], in_=rhs3_d.ap()[:, 0:HALF])
            nc.sync.dma_start(out=rhs3_sb[:, HALF:NLO], in_=rhs3_d.ap()[:, HALF:NLO])

            for p in range(NPASS):
                pt = pl.tile([128, NLO], f32, tag="c")
                for half in range(2):
                    nc.tensor.matmul(
                        out=pt[0:CT, half * HALF:(half + 1) * HALF],
                        lhsT=lhst_sb[:, p * CT:(p + 1) * CT],
                        rhs=rhs3_sb[:, half * HALF:(half + 1) * HALF],
                        start=True, stop=True)
                for half in range(2):
                    osb = wp.tile([CT, HALF], f16, tag=f"o{half}")
                    src = pt[0:CT, half * HALF:(half + 1) * HALF]
                    if half == 0:
                        nc.scalar.copy(out=osb[:], in_=src)
                    else:
                        nc.vector.tensor_copy(out=osb[:], in_=src)
                    nc.sync.dma_start(
                        out=out_v[:, p * NLO + half * HALF:p * NLO + (half + 1) * HALF],
                        in_=osb[:])

    nc.compile()
    return nc


def _in_maps(shuffle_vector: np.ndarray):
    tabs = _tables(shuffle_vector)
    return [{"rhs3": tabs["rhs3"], "lhst": np.ascontiguousarray(tabs["lhst"][c])}
            for c in range(NCORES)]


def _decode(results) -> np.ndarray:
    """Per-core out [102, 3072] fp16 -> normalized table (65536, 17) complex128."""
    blocks = []
    for c in range(NCORES):
        o = np.asarray(results[c]["out"]).astype(np.float64)
        o = o.reshape(NPASS, W2, NPASS, NLO)        # [j, dp, pass, lo]
        o = o.transpose(2, 0, 3, 1)                 # [pass, j, lo, dp]
        blocks.append(o.reshape(NPASS * NPASS, NLO, W2)[:GPC])
    allr = np.concatenate(blocks, 0).reshape(NHI * NLO, W2)
    tbl = allr[:, 0::2] + 1j * allr[:, 1::2]        # (65536, 17) complex128
    n2 = np.einsum("pd,pd->p", allr, allr)
    tbl *= (np.sqrt(M) / np.sqrt(n2))[:, None]
    return tbl


def kernel(x: np.ndarray, shuffle_vector: np.ndarray) -> np.ndarray:
    global _cached
    x = np.asarray(x)
    assert x.shape == (B, K), x.shape

    if _cached is None:
        _cached = _build_module()
    nc = _cached

    idx = ((x > 0).astype(np.uint32)
           @ (np.uint32(1) << np.arange(K, dtype=np.uint32)))
    res = bass_utils.run_bass_kernel_spmd(
        nc, _in_maps(shuffle_vector), core_ids=list(range(NCORES)))
    tbl = _decode(res.results)
    return tbl[idx]
